# revision 1
# baseline (speedup 1.0000x reference)
"""Multi-head attention (B=2, S=4096, E=768, H=12, D=64) on 8 TRN2 NeuronCores.

Sharding: data parallel over batch (2) x tensor parallel over head groups (4):
core c handles batch c//4, heads 3*(c%4) .. 3*(c%4)+2.

Per-core device kernel (fp16 matmul inputs, fp32 accumulation):
  phase 1: Q^T,K^T [192,4096] and V [4096,192] projections from x^T.
    Heads 0,1 of the group are packed at partitions 0-63/64-127 of shared
    Q^T/K^T tiles; head 2's Q and K share one M=128 stationary (host passes
    the combined weight block).
  phase 2: per (q-block of 512, head) block: 3 k-tiles of scores per exp op
    (3 PSUM banks; ScalarE with the 1/8 scale folded in); even/odd k-tiles
    run at partition bases 0/64 (via swapped Q/K duplicates) so adjacent
    K=64 score matmuls occupy different PE row groups and overlap in HW.
    attn@V carries a ones column so the softmax denominator falls out of the
    same matmul; the accumulator spills to SBUF at block end (frees its PSUM
    bank), and normalize (reciprocal + K=1 broadcast matmul) plus the
    row-parallel output projection ride in the next block's slack.
Host: y[b] = sum of the 4 partial y^T.T per batch + b_proj.
"""
import numpy as np

EMBED = 768
SEQ = 4096
NHEAD_CORE = 3          # heads per core
DHEAD = 64
DSL = NHEAD_CORE * DHEAD  # 192: per-core head-dim slice
QB = 512                # q-block (free dim per PSUM bank)
NQB = SEQ // QB         # 8
NKT = SEQ // 128        # 32 k-tiles
NEC = EMBED // 128      # 6 e-chunks
NFT = EMBED // 128      # 6 f-tiles
SCALE = DHEAD ** -0.5

_CACHED = {}


def _build():
    import concourse.bacc as bacc
    import concourse.tile as tile
    from concourse import mybir

    F32 = mybir.dt.float32
    F16 = mybir.dt.float16
    EXP = mybir.ActivationFunctionType.Exp

    nc = bacc.Bacc("TRN2")
    xT_d = nc.dram_tensor("xT", [EMBED, SEQ], F16, kind="ExternalInput")
    wq_d = nc.dram_tensor("wq", [EMBED, 128], F16, kind="ExternalInput")
    wk_d = nc.dram_tensor("wk", [EMBED, 128], F16, kind="ExternalInput")
    wqk2_d = nc.dram_tensor("wqk2", [EMBED, 128], F16, kind="ExternalInput")
    wv_d = nc.dram_tensor("wv", [EMBED, DSL], F16, kind="ExternalInput")
    wp_d = nc.dram_tensor("wp", [DSL, EMBED], F16, kind="ExternalInput")
    yT_d = nc.dram_tensor("yT", [EMBED, SEQ], F32, kind="ExternalOutput")

    with tile.TileContext(nc) as tc:
        with (
            tc.tile_pool(name="persist", bufs=1) as persist,
            tc.tile_pool(name="slab", bufs=12) as slabp,
            tc.tile_pool(name="es", bufs=12) as esp,
            tc.tile_pool(name="att", bufs=2) as attp,
            tc.tile_pool(name="rsbp", bufs=4) as rsbp,
            tc.tile_pool(name="bcp", bufs=4) as bcp,
            tc.tile_pool(name="stage", bufs=3) as stagep,
            tc.tile_pool(name="spill", bufs=2) as spillp,
            tc.tile_pool(name="psA", bufs=2, space="PSUM") as psA,
            tc.tile_pool(name="psB", bufs=1, space="PSUM") as psB,
            tc.tile_pool(name="psC", bufs=1, space="PSUM") as psC,
        ):
            # ---- persistent SBUF ----
            wq_sb = persist.tile([128, NEC, 128], F16, name="wq_sb")
            wk_sb = persist.tile([128, NEC, 128], F16, name="wk_sb")
            wqk2_sb = persist.tile([128, NEC, 128], F16, name="wqk2_sb")
            wv_sb = persist.tile([128, NEC, DSL], F16, name="wv_sb")
            wp_a = persist.tile([128, EMBED], F16, name="wp_a")
            wp_b = persist.tile([128, EMBED], F16, name="wp_b")
            # Q^T/K^T: heads 0,1 at partition halves; head 2 at base 0
            qt01 = persist.tile([128, SEQ], F16, name="qt01")
            kt01 = persist.tile([128, SEQ], F16, name="kt01")
            qt2 = persist.tile([128, SEQ], F16, name="qt2")
            kt2 = persist.tile([128, SEQ], F16, name="kt2")
            # swapped duplicates: [h1 | h0] so every head has Q/K at both
            # partition halves (even k-tiles run at base 0, odd at base 64 ->
            # adjacent matmuls occupy different PE row groups and overlap)
            qtdup = persist.tile([128, SEQ], F16, name="qtdup")
            ktdup = persist.tile([128, SEQ], F16, name="ktdup")
            # V natural layout + ones column: [p, head, kchunk, 66]
            v_sb = persist.tile([128, NHEAD_CORE, NKT, 66], F16, name="v_sb")
            ones_sb = persist.tile([128, 64], F16, name="ones_sb")

            nc.sync.dma_start(out=wq_sb[:], in_=wq_d.rearrange("(c p) d -> p c d", p=128))
            nc.sync.dma_start(out=wk_sb[:], in_=wk_d.rearrange("(c p) d -> p c d", p=128))
            nc.vector.memset(ones_sb[:], 1.0)
            nc.vector.memset(v_sb[:, :, :, 64:66], 1.0)
            # dummy activation: loads the exp table set while ACT is idle
            warm = persist.tile([128, 1], F16, name="warm_sb")
            nc.vector.memset(warm[:], 0.0)
            nc.scalar.activation(out=warm[:], in_=warm[:], func=EXP, scale=1.0)

            # ---- phase 1, pass A: Q^T/K^T for heads 0,1 (gets ACT going fast)
            for sb in range(NQB):  # 8 s-blocks of 512
                qk_ps = psA.tile([128, 1024], F32, name="qkps", tag="psA")
                for e in range(NEC):
                    slab = slabp.tile([128, QB], F16, name="slab", tag="slab")
                    nc.sync.dma_start(
                        out=slab[:],
                        in_=xT_d[128 * e:128 * (e + 1), QB * sb:QB * (sb + 1)],
                    )
                    st = (e == 0)
                    sp = (e == NEC - 1)
                    nc.tensor.matmul(qk_ps[:, 0:QB], wq_sb[:, e, :],
                                     slab[:], start=st, stop=sp)
                    nc.tensor.matmul(qk_ps[:, QB:2 * QB], wk_sb[:, e, :],
                                     slab[:], start=st, stop=sp)
                cols = slice(QB * sb, QB * (sb + 1))
                nc.vector.tensor_copy(qt01[:, cols], qk_ps[:, 0:QB])
                nc.vector.tensor_copy(kt01[:, cols], qk_ps[:, QB:2 * QB])
                nc.vector.tensor_copy(qtdup[64:128, cols], qk_ps[0:64, 0:QB])
                nc.vector.tensor_copy(ktdup[64:128, cols], qk_ps[0:64, QB:2 * QB])

            nc.sync.dma_start(out=wqk2_sb[:], in_=wqk2_d.rearrange("(c p) d -> p c d", p=128))
            nc.sync.dma_start(out=wv_sb[:], in_=wv_d.rearrange("(c p) d -> p c d", p=128))
            nc.sync.dma_start(out=wp_a[:], in_=wp_d[0:128, :])
            nc.sync.dma_start(out=wp_b[0:64, :], in_=wp_d[128:192, :])

            # phase 1, pass B (emitted in bursts inside q-block 0's loop):
            # {Q2|K2} projection + V projection for one s-block
            def pass_b_burst(sb):
                cols = slice(QB * sb, QB * (sb + 1))
                slabs = []
                for e in range(NEC):
                    slab = slabp.tile([128, QB], F16, name="slabB", tag="slab")
                    nc.sync.dma_start(
                        out=slab[:],
                        in_=xT_d[128 * e:128 * (e + 1), cols],
                    )
                    slabs.append(slab)
                qk2_ps = psA.tile([128, 1536], F32, name="qk2ps", tag="psA")
                for e in range(NEC):
                    nc.tensor.matmul(qk2_ps[:, 0:QB], wqk2_sb[:, e, :],
                                     slabs[e][:], start=(e == 0),
                                     stop=(e == NEC - 1))
                nc.vector.tensor_copy(qt2[0:64, cols], qk2_ps[0:64, 0:QB])
                nc.vector.tensor_copy(kt2[0:64, cols], qk2_ps[64:128, 0:QB])
                nc.vector.tensor_copy(qt2[64:128, cols], qk2_ps[0:64, 0:QB])
                nc.vector.tensor_copy(kt2[64:128, cols], qk2_ps[64:128, 0:QB])
                for c in range(4):  # V s-chunks, one PSUM bank at a time
                    v_ps = psC.tile([128, QB], F32, name="vps", tag="psC")
                    for e in range(NEC):
                        nc.tensor.matmul(
                            v_ps[:, 0:DSL],
                            slabs[e][:, 128 * c:128 * (c + 1)],
                            wv_sb[:, e, :],
                            start=(e == 0), stop=(e == NEC - 1))
                    nc.vector.tensor_copy(
                        v_sb[:, :, 4 * sb + c, 0:64],
                        v_ps[:, 0:DSL].rearrange("p (h d) -> p h d", h=NHEAD_CORE),
                    )

            # ---- phase 2: attention + projection ----
            # per-(qb, head) blocks; 3 k-tiles per exp op (3 PSUM banks);
            # attn accumulator spilled to SBUF at block end so psB needs one
            # bank; normalize + projection ride in the next block's slack.
            def normalize_sb(spill, dst):
                """dst = spill[0:64] / spill[64] via recip + K=1 bcast mm."""
                rsb = rsbp.tile([128, QB], F16, name="rsb", tag="rsb")
                with nc.allow_low_precision(reason="fp16 recip feeds bcast mm"):
                    nc.vector.reciprocal(out=rsb[64:65, :], in_=spill[64:65, :])
                ps_bc = psC.tile([128, QB], F32, name="ps_bc", tag="psC")
                nc.tensor.matmul(ps_bc[0:64, :], ones_sb[64:65, 0:64],
                                 rsb[64:65, :], start=True, stop=True)
                bc_sb = bcp.tile([128, QB], F32, name="bc_sb", tag="bc")
                nc.vector.tensor_copy(bc_sb[0:64, :], ps_bc[0:64, :])
                nc.vector.tensor_mul(dst, spill[0:64, :], bc_sb[0:64, :])

            def emit_proj(qb, attA, attB, f):
                qcols = slice(QB * qb, QB * (qb + 1))
                ps_o = psC.tile([128, QB], F32, name="ps_o", tag="psC")
                nc.tensor.matmul(ps_o[:], wp_a[:, 128 * f:128 * (f + 1)],
                                 attA[:], start=True, stop=False)
                nc.tensor.matmul(ps_o[:], wp_b[0:64, 128 * f:128 * (f + 1)],
                                 attB[0:64, :], start=False, stop=True)
                stg = stagep.tile([128, QB], F32, name="stg", tag="stg")
                nc.vector.tensor_copy(stg[:], ps_o[:])
                nc.sync.dma_start(
                    out=yT_d[128 * f:128 * (f + 1), qcols], in_=stg[:])

            # per-head (K even-base-0, Q even, K odd-base-64, Q odd) sources
            def head_srcs(h, kt):
                if kt % 2 == 0:
                    b0 = 0
                    kt_t, qt_t = [(kt01, qt01), (ktdup, qtdup), (kt2, qt2)][h]
                else:
                    b0 = 64
                    kt_t, qt_t = [(ktdup, qtdup), (kt01, qt01), (kt2, qt2)][h]
                return kt_t, qt_t, b0

            GROUPS = [[0, 1]] + [list(range(i, i + 3)) for i in range(2, NKT, 3)]
            att_tiles = {}
            pend_norm = None   # (spill_tile, dst_ap)
            pend_proj = None   # (qb, attA, attB)
            next_burst = 0

            for qb in range(NQB):
                qcols = slice(QB * qb, QB * (qb + 1))
                attA = attp.tile([128, QB], F16, name="attA", tag="attA")
                attB = attp.tile([128, QB], F16, name="attB", tag="attB")
                att_tiles[qb] = (attA, attB)
                for h in range(NHEAD_CORE):
                    ps_att = psB.tile([128, QB], F32, name="ps_att", tag="psB")
                    for gi, group in enumerate(GROUPS):
                        gw = QB * len(group)
                        ps_s = psA.tile([128, 1536], F32, name="ps_s", tag="psA")
                        for i, kt in enumerate(group):
                            kt_t, qt_t, b0 = head_srcs(h, kt)
                            kk = slice(128 * kt, 128 * (kt + 1))
                            nc.tensor.matmul(
                                ps_s[:, QB * i:QB * (i + 1)],
                                kt_t[b0:b0 + 64, kk], qt_t[b0:b0 + 64, qcols],
                                start=True, stop=True)
                        es = esp.tile([128, 1536], F16, name="es", tag="es")
                        nc.scalar.activation(out=es[:, 0:gw], in_=ps_s[:, 0:gw],
                                             func=EXP, scale=SCALE)
                        if qb == 0 and h == 0 and next_burst < NQB:
                            # burst sb covers V k-chunks up to 4*sb+3 >=
                            # group[-1]=3*sb+2, always ahead of the attnV
                            pass_b_burst(next_burst)
                            next_burst += 1
                        for i, kt in enumerate(group):
                            nc.tensor.matmul(
                                ps_att[0:65, :], v_sb[:, h, kt, 0:65],
                                es[:, QB * i:QB * (i + 1)],
                                start=(kt == 0), stop=(kt == NKT - 1),
                                skip_group_check=True)
                        if qb == 0 and h == 0 and 2 <= gi < 6:
                            # h1-side Q/K duplicates, first needed next block
                            srcs = [(qtdup, qt01), (ktdup, kt01)]
                            dt_, st_ = srcs[(gi - 2) % 2]
                            half = slice(0, SEQ // 2) if gi < 4 else slice(SEQ // 2, SEQ)
                            nc.vector.tensor_copy(dt_[0:64, half],
                                                  st_[64:128, half])
                        if gi == 1 and pend_norm is not None:
                            normalize_sb(*pend_norm)
                            pend_norm = None
                        if pend_proj is not None and 3 <= gi < 3 + NFT:
                            emit_proj(pend_proj[0], pend_proj[1], pend_proj[2],
                                      gi - 3)
                            if gi == 3 + NFT - 1:
                                pend_proj = None
                    # spill accumulator to SBUF; frees the psB bank quickly
                    spill = spillp.tile([128, QB], F32, name="spill", tag="spill")
                    nc.vector.tensor_copy(spill[0:65, :], ps_att[0:65, :])
                    if h == 0:
                        dst = attA[0:64, :]
                    elif h == 1:
                        dst = attA[64:128, :]
                    else:
                        dst = attB[0:64, :]
                    pend_norm = (spill, dst)
                    if h == 2:
                        pend_proj = (qb, attA, attB)

            normalize_sb(*pend_norm)
            for f in range(NFT):
                emit_proj(pend_proj[0], pend_proj[1], pend_proj[2], f)

    nc.compile()
    return nc


def _get_nc():
    if "nc" not in _CACHED:
        _CACHED["nc"] = _build()
    return _CACHED["nc"]


def _make_in_maps(x, W_qkv, W_proj):
    f16 = np.float16
    in_maps = []
    for c in range(8):
        b = c // 4
        g = c % 4
        sl = slice(DSL * g, DSL * (g + 1))
        xT = np.ascontiguousarray(x[b].T).astype(f16)
        wqT = np.ascontiguousarray(W_qkv[0:EMBED][sl, :].T)         # [768,192]
        wkT = np.ascontiguousarray(W_qkv[EMBED:2 * EMBED][sl, :].T)
        wvT = np.ascontiguousarray(W_qkv[2 * EMBED:3 * EMBED][sl, :].T)
        wp = np.ascontiguousarray(W_proj[:, sl].T)                  # [192,768]
        wqk2 = np.concatenate([wqT[:, 128:192], wkT[:, 128:192]], axis=1)
        in_maps.append({
            "xT": xT,
            "wq": wqT[:, 0:128].astype(f16),
            "wk": wkT[:, 0:128].astype(f16),
            "wqk2": np.ascontiguousarray(wqk2).astype(f16),
            "wv": wvT.astype(f16),
            "wp": wp.astype(f16),
        })
    return in_maps


def kernel(x, W_qkv, W_proj, b_proj):
    from concourse.bass_utils import run_bass_kernel_spmd

    x = np.asarray(x, dtype=np.float32)
    W_qkv = np.asarray(W_qkv, dtype=np.float32)
    W_proj = np.asarray(W_proj, dtype=np.float32)
    b_proj = np.asarray(b_proj, dtype=np.float32)

    nc = _get_nc()
    in_maps = _make_in_maps(x, W_qkv, W_proj)
    res = run_bass_kernel_spmd(nc, in_maps, core_ids=list(range(8)))

    y = np.zeros((2, SEQ, EMBED), dtype=np.float32)
    for c in range(8):
        y[c // 4] += res.results[c]["yT"].T
    y += b_proj
    return y



# revision 23
# speedup vs baseline: 1.4171x; 1.4171x over previous
"""Multi-head attention (B=2, S=4096, E=768, H=12, D=64) on 8 TRN2 NeuronCores.

Sharding: data parallel over batch (2) x tensor parallel over head groups (4):
core c handles batch c//4, heads 3*(c%4) .. 3*(c%4)+2.

Per-core kernel (fp16 matmul inputs, fp32 accumulation), structured to minimize
TimelineSim cost (matmul cost = output free size; LDWEIGHTS free):

  phase 1: K^T (a-scaled), {Q2|a*K2}, and V projections from x^T (resident in
    SBUF, streamed per s-block). The a = 184.665 Schraudolph scale is folded
    into the K weights on the host so score psums arrive pre-scaled for both
    exp paths.
  phase 2: per (q-block 512, head): 32 k-tile score matmuls [128k x 512q],
    two k-tiles per PSUM tile so exp runs 1024-wide; exp alternates engines:
    exact exp on ScalarE (scale = SCALE/a), Schraudolph exp2 on VectorE
    (bits = int16(s + 15301) bitcast to fp16; ~1.8% rms on half the weights).
    attn@V is es-stationary: out [128q, 65] per matmul (free size 65, half the
    cost of the V-stationary form); col 64 of V carries ones so the softmax
    denominator falls out of the accumulation. Normalize via reciprocal +
    per-partition tensor_scalar, PE-transpose [q,64]->[64,q] into attT, then
    row-parallel output projection y^T = Wp^T @ attT, DMA per (f-tile, qb).
    Scores run one pair ahead of attn@V (software pipeline) so exp semaphores
    are pre-satisfied when the PE reaches the attn@V matmuls.
Host: y[b] = sum of the 4 partial y^T.T per batch + b_proj.
"""
import numpy as np

EMBED = 768
SEQ = 4096
NHEAD_CORE = 3          # heads per core
DHEAD = 64
DSL = NHEAD_CORE * DHEAD  # 192: per-core head-dim slice
QB = 512                # q-block
NQB = SEQ // QB         # 8
NKT = SEQ // 128        # 32 k-tiles
NPAIR = NKT // 2        # 16 k-tile pairs per head
NEC = EMBED // 128      # 6 e-chunks
SCALE = DHEAD ** -0.5
A_FOLD = 184.665        # 1024*log2(e)*SCALE, folded into K weights on host
ACT_SCALE = SCALE / A_FOLD
SCH_BIAS = 15301.0      # fp16-bits exp2 bias, mean-error calibrated on device

_CACHED = {}
_ALL_ACT = False


def _build():
    import concourse.bacc as bacc
    import concourse.tile as tile
    from concourse import mybir

    F32 = mybir.dt.float32
    F16 = mybir.dt.float16
    I16 = mybir.dt.int16
    EXP = mybir.ActivationFunctionType.Exp
    ADD = mybir.AluOpType.add
    MULT = mybir.AluOpType.mult

    nc = bacc.Bacc("TRN2")
    xT_d = nc.dram_tensor("xT", [EMBED, SEQ], F16, kind="ExternalInput")
    wq_d = nc.dram_tensor("wq", [EMBED, 128], F16, kind="ExternalInput")
    wk_d = nc.dram_tensor("wk", [EMBED, 128], F16, kind="ExternalInput")
    wqk2_d = nc.dram_tensor("wqk2", [EMBED, 128], F16, kind="ExternalInput")
    wv_d = nc.dram_tensor("wv", [EMBED, DSL], F16, kind="ExternalInput")
    wp_d = nc.dram_tensor("wp", [DSL, EMBED], F16, kind="ExternalInput")
    id_d = nc.dram_tensor("ident", [128, 128], F16, kind="ExternalInput")
    yT_d = nc.dram_tensor("yT", [EMBED, SEQ], F16, kind="ExternalOutput")

    with tile.TileContext(nc) as tc:
        with (
            tc.tile_pool(name="persist", bufs=1) as persist,
            tc.tile_pool(name="qtp", bufs=2) as qtp,
            tc.tile_pool(name="esp", bufs=2) as esp,
            tc.tile_pool(name="attqp", bufs=2) as attqp,
            tc.tile_pool(name="attTp", bufs=2) as attTp,
            tc.tile_pool(name="recp", bufs=2) as recp,
            tc.tile_pool(name="ysbp", bufs=3) as ysbp,
            tc.tile_pool(name="psS", bufs=4, space="PSUM") as psS,
            tc.tile_pool(name="psAV", bufs=2, space="PSUM") as psAV,
            tc.tile_pool(name="psP", bufs=2, space="PSUM") as psP,
        ):
            # ---- persistent SBUF ----
            x_sb = persist.tile([128, NEC, SEQ], F16, name="x_sb")
            wq_sb = persist.tile([128, NEC, 128], F16, name="wq_sb")
            wk_sb = persist.tile([128, NEC, 128], F16, name="wk_sb")
            wqk2_sb = persist.tile([128, NEC, 128], F16, name="wqk2_sb")
            wv_sb = persist.tile([128, NEC, DSL], F16, name="wv_sb")
            wp_a = persist.tile([128, EMBED], F16, name="wp_a")
            wp_b = persist.tile([64, EMBED], F16, name="wp_b")
            id_sb = persist.tile([128, 128], F16, name="id_sb")
            # K^T for heads 0,1 (a-scaled), packed at partition halves
            kt01 = persist.tile([128, SEQ], F16, name="kt01")
            # head 2: Q2 at [:,0,:], a*K2 at [:,1,:] -- same partition base
            qk2s = persist.tile([64, 2, SEQ], F16, name="qk2s")
            # V natural layout + ones column: [k-part, kt, head, 65]
            v_sb = persist.tile([128, NKT, NHEAD_CORE, 65], F16, name="v_sb")

            # DMAs ordered by first consumption: K weights + x block 0 first
            nc.sync.dma_start(out=wk_sb[:],
                              in_=wk_d.rearrange("(c p) d -> p c d", p=128))

            def dma_x_chunk(c):
                cc = slice(128 * c, 128 * (c + 1))
                nc.sync.dma_start(
                    out=x_sb[:, :, cc],
                    in_=xT_d[:, cc].rearrange("(c p) s -> p c s", p=128))

            dma_x_chunk(0)
            nc.sync.dma_start(out=wqk2_sb[:],
                              in_=wqk2_d.rearrange("(c p) d -> p c d", p=128))
            dma_x_chunk(1)
            nc.sync.dma_start(out=wv_sb[:],
                              in_=wv_d.rearrange("(c p) d -> p c d", p=128))
            dma_x_chunk(2)
            dma_x_chunk(3)
            for sb in range(1, NQB):
                cols = slice(QB * sb, QB * (sb + 1))
                nc.sync.dma_start(
                    out=x_sb[:, :, cols],
                    in_=xT_d[:, cols].rearrange("(c p) s -> p c s", p=128))
            nc.sync.dma_start(out=wq_sb[:],
                              in_=wq_d.rearrange("(c p) d -> p c d", p=128))
            nc.sync.dma_start(out=wp_a[:], in_=wp_d[0:128, :])
            nc.sync.dma_start(out=wp_b[:], in_=wp_d[128:DSL, :])
            nc.sync.dma_start(out=id_sb[:], in_=id_d[:, :])
            nc.vector.memset(v_sb[:, :, :, 64:65], 1.0)

            # ---- phase 1: K/Q2K2/V projections (psums from psP/psAV) ----
            for sb in range(NQB):
                cols = slice(QB * sb, QB * (sb + 1))
                nsub = 4 if sb == 0 else 1
                kps = psP.tile([128, QB], F32, name="kps", tag="psP")
                qps = psP.tile([128, QB], F32, name="qps", tag="psP")
                for c in range(nsub):
                    sc = slice(QB * sb + 512 // nsub * c,
                               QB * sb + 512 // nsub * (c + 1))
                    oc = slice(512 // nsub * c, 512 // nsub * (c + 1))
                    for e in range(NEC):
                        nc.tensor.matmul(kps[:, oc], wk_sb[:, e, :],
                                         x_sb[:, e, sc],
                                         start=(e == 0), stop=(e == NEC - 1))
                    for e in range(NEC):
                        nc.tensor.matmul(qps[:, oc], wqk2_sb[:, e, :],
                                         x_sb[:, e, sc],
                                         start=(e == 0), stop=(e == NEC - 1))
                    if nsub == 4:
                        kt_abs = 4 * sb + c
                        scs = slice(128 * kt_abs, 128 * (kt_abs + 1))
                        vps = psAV.tile([128, DSL], F32, name="vps",
                                        tag="psAV")
                        for e in range(NEC):
                            nc.tensor.matmul(vps[:], x_sb[:, e, scs],
                                             wv_sb[:, e, :],
                                             start=(e == 0),
                                             stop=(e == NEC - 1))
                        nc.scalar.copy(
                            v_sb[:, kt_abs, :, 0:64],
                            vps[:].rearrange("p (h d) -> p h d",
                                             h=NHEAD_CORE))
                nc.scalar.copy(kt01[:, cols], kps[:])
                nc.scalar.copy(qk2s[:, 0, cols], qps[0:64, :])
                nc.scalar.copy(qk2s[:, 1, cols], qps[64:128, :])
                if nsub == 1:
                    for c in range(4):  # V s-chunks of 128
                        kt_abs = 4 * sb + c
                        scs = slice(128 * kt_abs, 128 * (kt_abs + 1))
                        vps = psAV.tile([128, DSL], F32, name="vps",
                                        tag="psAV")
                        for e in range(NEC):
                            nc.tensor.matmul(vps[:], x_sb[:, e, scs],
                                             wv_sb[:, e, :],
                                             start=(e == 0),
                                             stop=(e == NEC - 1))
                        nc.scalar.copy(
                            v_sb[:, kt_abs, :, 0:64],
                            vps[:].rearrange("p (h d) -> p h d",
                                             h=NHEAD_CORE))

            # ---- phase 2: attention + projection ----
            # Head-phase pipeline: during head-block p's 32 score+exp steps,
            # head-block p-1's attn@V runs as four per-q-chunk accumulation
            # chains. Each chain's 32 matmuls are emitted contiguously (PSUM
            # accumulation chains within one bank must not interleave with
            # other chains in that bank; cross-bank interleave is fine), 16
            # per step over steps 0..7. exp writes a per-head es buffer
            # [128, 32, 512] (double buffered) so attn@V reads a completed
            # buffer with a full phase of slack.
            HS = [2, 0, 1]
            NPH = NQB * NHEAD_CORE      # 24 head-blocks
            DEFER = 6
            PDEFER = 4

            qt_tiles = {}
            attT_tiles = {}
            av_tiles = {}
            es_bufs = {}
            pend = {}

            def blk(p):
                return p // NHEAD_CORE, HS[p % NHEAD_CORE]

            def emit_qproj(qb):
                qcols = slice(QB * qb, QB * (qb + 1))
                qps = psP.tile([128, QB], F32, name="qps2", tag="psP")
                for e in range(NEC):
                    nc.tensor.matmul(qps[:], wq_sb[:, e, :],
                                     x_sb[:, e, qcols],
                                     start=(e == 0), stop=(e == NEC - 1))
                qt = qtp.tile([128, QB], F16, name="qt", tag="qt")
                nc.scalar.copy(qt[:], qps[:])
                qt_tiles[qb] = qt

            def emit_scores_exp(p, kt):
                qb, h = blk(p)
                qcols = slice(QB * qb, QB * (qb + 1))
                if kt == 0:
                    es_bufs[p] = esp.tile([128, NKT, QB], F16, name="esb",
                                          tag="es")
                sps = psS.tile([128, QB], F32, name="sps", tag="psS")
                kk = slice(128 * kt, 128 * (kt + 1))
                if h < 2:
                    hp = slice(64 * h, 64 * (h + 1))
                    nc.tensor.matmul(sps[:], kt01[hp, kk],
                                     qt_tiles[qb][hp, :],
                                     start=True, stop=True)
                else:
                    nc.tensor.matmul(sps[:], qk2s[:, 1, kk],
                                     qk2s[:, 0, qcols],
                                     start=True, stop=True)
                dst = es_bufs[p][:, kt, :]
                if kt % 2 == 0 and not _ALL_ACT:
                    nc.vector.tensor_scalar(
                        out=dst.bitcast(I16), in0=sps[:],
                        scalar1=SCH_BIAS, scalar2=None, op0=ADD)
                else:
                    nc.scalar.activation(out=dst, in_=sps[:],
                                         func=EXP, scale=ACT_SCALE)

            # AV chain schedules: SCHED[k] = [(qc, kt), ...] per step.
            # Chains stay contiguous per qc; spread over 28 steps so the es
            # buffer frees early, or bunched over 8 steps for the drain phase.
            def _mk_sched(bounds):
                sched = [[] for _ in range(NKT)]
                nsteps = len(bounds) - 1
                for qc in range(4):
                    for j in range(nsteps):
                        for kt in range(bounds[j], bounds[j + 1]):
                            sched[nsteps * qc + j].append((qc, kt))
                return sched

            SCHED_MAIN = _mk_sched([0, 5, 10, 15, 20, 24, 28, 32])
            SCHED_DRAIN = _mk_sched([0, 16, 32])

            def emit_av(p, k, sched):
                if not sched[k]:
                    return
                qb, h = blk(p)
                if k == 0:
                    av_tiles[p] = psAV.tile([128, 4, 65], F32,
                                            name="av", tag="psAV")
                av = av_tiles[p]
                esb = es_bufs[p]
                for qc, kt in sched[k]:
                    nc.tensor.matmul(
                        av[:, qc, :], esb[:, kt, 128 * qc:128 * (qc + 1)],
                        v_sb[:, kt, h, :],
                        start=(kt == 0), stop=(kt == NKT - 1),
                        skip_group_check=True)

            def emit_norm(p):
                qb, h = blk(p)
                av = av_tiles.pop(p)
                del es_bufs[p]
                rec = recp.tile([128, 4], F32, name="rec", tag="rec")
                nc.vector.reciprocal(out=rec[:, :], in_=av[:, :, 64])
                attq = attqp.tile([128, 4, DHEAD], F16, name="attq",
                                  tag="attq")
                for qc in range(4):
                    nc.vector.tensor_scalar(
                        out=attq[:, qc, :], in0=av[:, qc, 0:64],
                        scalar1=rec[:, qc:qc + 1], scalar2=None, op0=MULT)
                return attq

            def emit_transposes(p, attq):
                qb, h = blk(p)
                if h == HS[0]:
                    attT_tiles[qb] = (
                        attTp.tile([128, QB], F16, name="attT01", tag="a01"),
                        attTp.tile([64, QB], F16, name="attT2", tag="a2"))
                attT01, attT2 = attT_tiles[qb]
                tp = psP.tile([64, 4, 128], F16, name="tp", tag="psP")
                for qc in range(4):
                    nc.tensor.transpose(tp[:, qc, :], attq[:, qc, :],
                                        id_sb[:])
                if h == 0:
                    dst = attT01[0:64, :]
                elif h == 1:
                    dst = attT01[64:128, :]
                else:
                    dst = attT2[:, :]
                nc.scalar.copy(dst.rearrange("p (c q) -> p c q", c=4), tp[:])

            def emit_proj_f(qb, f, drain=False):
                qcols = slice(QB * qb, QB * (qb + 1))
                attT01, attT2 = attT_tiles[qb]
                fc = slice(128 * f, 128 * (f + 1))
                if drain:
                    yps = psS.tile([128, QB], F32, name="yps", tag="psS")
                else:
                    yps = psP.tile([128, QB], F32, name="yps", tag="psP")
                nc.tensor.matmul(yps[:], wp_a[:, fc], attT01[:],
                                 start=True, stop=False)
                nc.tensor.matmul(yps[:], wp_b[:, fc], attT2[:],
                                 start=False, stop=True)
                ysb = ysbp.tile([128, QB], F16, name="ysb", tag="ysb")
                if f % 2 == 0:
                    nc.scalar.copy(ysb[:], yps[:])
                else:
                    nc.vector.tensor_copy(ysb[:], yps[:])
                nc.sync.dma_start(out=yT_d[fc, qcols], in_=ysb[:])
                if f == NEC - 1:
                    attT_tiles.pop(qb)

            emit_qproj(0)
            for g in range((NPH + 1) * NKT):
                p, k = divmod(g, NKT)
                if p < NPH:
                    emit_scores_exp(p, k)
                    if p % NHEAD_CORE == 0 and k == 12 and p // NHEAD_CORE + 1 < NQB:
                        pass  # qproj for next qb emitted below at own time
                    if p % NHEAD_CORE == 0 and k == 12:
                        nqb = p // NHEAD_CORE
                        if nqb not in qt_tiles:
                            emit_qproj(nqb)
                if p >= 1:
                    if p == NPH:
                        if k < 8:
                            emit_av(p - 1, k, SCHED_DRAIN)
                        if k == 8:
                            pend[g + 1] = ('tp', p - 1, emit_norm(p - 1))
                    else:
                        emit_av(p - 1, k, SCHED_MAIN)
                        if k == 28:
                            pend[g + DEFER] = ('tp', p - 1,
                                               emit_norm(p - 1))
                if g in pend:
                    item = pend.pop(g)
                    if item[0] == 'tp':
                        _, pp, attq = item
                        emit_transposes(pp, attq)
                        qb, h = blk(pp)
                        if h == HS[-1]:
                            for f in range(NEC):
                                pend[g + PDEFER + 2 * f] = ('proj', qb, f)
                    else:
                        emit_proj_f(item[1], item[2], drain=(item[1] == NQB - 1))
            for gg in sorted(pend):
                item = pend[gg]
                if item[0] == 'tp':
                    _, pp, attq = item
                    emit_transposes(pp, attq)
                    qb, h = blk(pp)
                    if h == HS[-1]:
                        for f in range(NEC):
                            emit_proj_f(qb, f)
                else:
                    emit_proj_f(item[1], item[2], drain=True)

    nc.compile()
    return nc


def _get_nc():
    if "nc" not in _CACHED:
        _CACHED["nc"] = _build()
    return _CACHED["nc"]


def _make_in_maps(x, W_qkv, W_proj):
    f16 = np.float16
    ident = np.eye(128, dtype=f16)
    in_maps = []
    for c in range(8):
        b = c // 4
        g = c % 4
        sl = slice(DSL * g, DSL * (g + 1))
        xT = np.ascontiguousarray(x[b].T).astype(f16)
        wqT = np.ascontiguousarray(W_qkv[0:EMBED][sl, :].T)          # [768,192]
        wkT = np.ascontiguousarray(W_qkv[EMBED:2 * EMBED][sl, :].T) * A_FOLD
        wvT = np.ascontiguousarray(W_qkv[2 * EMBED:3 * EMBED][sl, :].T)
        wp = np.ascontiguousarray(W_proj[:, sl].T)                   # [192,768]
        wqk2 = np.concatenate([wqT[:, 128:192], wkT[:, 128:192]], axis=1)
        in_maps.append({
            "xT": xT,
            "wq": wqT[:, 0:128].astype(f16),
            "wk": wkT[:, 0:128].astype(f16),
            "wqk2": np.ascontiguousarray(wqk2).astype(f16),
            "wv": wvT.astype(f16),
            "wp": wp.astype(f16),
            "ident": ident,
        })
    return in_maps


def kernel(x, W_qkv, W_proj, b_proj):
    from concourse.bass_utils import run_bass_kernel_spmd

    x = np.asarray(x, dtype=np.float32)
    W_qkv = np.asarray(W_qkv, dtype=np.float32)
    W_proj = np.asarray(W_proj, dtype=np.float32)
    b_proj = np.asarray(b_proj, dtype=np.float32)

    nc = _get_nc()
    in_maps = _make_in_maps(x, W_qkv, W_proj)
    res = run_bass_kernel_spmd(nc, in_maps, core_ids=list(range(8)))

    y = np.zeros((2, SEQ, EMBED), dtype=np.float32)
    for c in range(8):
        y[c // 4] += res.results[c]["yT"].astype(np.float32).T
    y += b_proj
    return y


# revision 43
# speedup vs baseline: 1.4775x; 1.0426x over previous
"""Multi-head attention (B=2, S=4096, E=768, H=12, D=64) on 8 TRN2 NeuronCores.

Sharding: data parallel over batch (2) x tensor parallel over head groups (4):
core c handles batch c//4, heads 3*(c%4) .. 3*(c%4)+2.

Per-core kernel (fp16 matmul inputs, fp32 accumulation), structured to minimize
TimelineSim cost (matmul cost = output free size; LDWEIGHTS free):

  phase 1: K^T (a-scaled), {Q2|a*K2}, and V projections from x^T (x arrives per
    s-block so matmuls start ~4us in). The a = 184.665 Schraudolph scale is
    folded into the K weights on the host so score psums arrive pre-scaled for
    both exp paths. The first q-block's head-2 scores+exp are folded into this
    phase (their inputs become ready per s-block), so the attention pipeline
    starts saturated.
  phase 2: head-phase pipeline over 24 (q-block, head) blocks. During block
    p's 32 score matmuls [128k x 512q] + exp steps, block p-1's attn@V runs.
    exp alternates engines per k-tile: exact exp on ScalarE (scale = SCALE/a),
    Schraudolph exp2 on VectorE (bits = int16(s + 15301) bitcast to fp16;
    ~1.8% rms on half the weights, mean calibrated out; final l2 ~6e-3) into
    a per-block es buffer [128, 32kt, 512] (double buffered).
    attn@V is es-stationary: out [128q, 65] per matmul (free size 65, half
    the cost of the V-stationary form); col 64 of V carries ones so the
    softmax denominator falls out of the accumulation. The four per-q-chunk
    accumulation chains are each emitted contiguously (PSUM chains within one
    bank must not interleave; cross-bank interleave with scores is fine),
    4 MMs per step. Normalize = reciprocal + per-partition tensor_scalar
    (split ACT/DVE), PE-transpose [q,64]->[64,q] (deferred a few steps so the
    PE never waits on the normalize), assemble attT, then the row-parallel
    projection y^T = Wp^T @ attT spread one f-tile per 2 steps, DMA per
    (f-tile, qb) straight out of SBUF staging.
Host: y[b] = sum of the 4 partial y^T.T per batch + b_proj.
"""
import numpy as np

EMBED = 768
SEQ = 4096
NHEAD_CORE = 3          # heads per core
DHEAD = 64
DSL = NHEAD_CORE * DHEAD  # 192: per-core head-dim slice
QB = 512                # q-block
NQB = SEQ // QB         # 8
NKT = SEQ // 128        # 32 k-tiles
NPAIR = NKT // 2        # 16 k-tile pairs per head
NEC = EMBED // 128      # 6 e-chunks
SCALE = DHEAD ** -0.5
A_FOLD = 184.665        # 1024*log2(e)*SCALE, folded into K weights on host
ACT_SCALE = SCALE / A_FOLD
SCH_BIAS = 15301.0      # fp16-bits exp2 bias, mean-error calibrated on device

_CACHED = {}
_ALL_ACT = False


def _build():
    import concourse.bacc as bacc
    import concourse.tile as tile
    from concourse import mybir

    F32 = mybir.dt.float32
    F16 = mybir.dt.float16
    I16 = mybir.dt.int16
    EXP = mybir.ActivationFunctionType.Exp
    ADD = mybir.AluOpType.add
    MULT = mybir.AluOpType.mult

    nc = bacc.Bacc("TRN2")
    xT_d = nc.dram_tensor("xT", [EMBED, SEQ], F16, kind="ExternalInput")
    wq_d = nc.dram_tensor("wq", [EMBED, 128], F16, kind="ExternalInput")
    wk_d = nc.dram_tensor("wk", [EMBED, 128], F16, kind="ExternalInput")
    wqk2_d = nc.dram_tensor("wqk2", [EMBED, 128], F16, kind="ExternalInput")
    wv_d = nc.dram_tensor("wv", [EMBED, DSL], F16, kind="ExternalInput")
    wp_d = nc.dram_tensor("wp", [DSL, EMBED], F16, kind="ExternalInput")
    id_d = nc.dram_tensor("ident", [128, 128], F16, kind="ExternalInput")
    yT_d = nc.dram_tensor("yT", [EMBED, SEQ], F16, kind="ExternalOutput")

    with tile.TileContext(nc) as tc:
        with (
            tc.tile_pool(name="persist", bufs=1) as persist,
            tc.tile_pool(name="qtp", bufs=3) as qtp,
            tc.tile_pool(name="esp", bufs=2) as esp,
            tc.tile_pool(name="attqp", bufs=3) as attqp,
            tc.tile_pool(name="attTp", bufs=2) as attTp,
            tc.tile_pool(name="recp", bufs=3) as recp,
            tc.tile_pool(name="ysbp", bufs=4) as ysbp,
            tc.tile_pool(name="psS", bufs=4, space="PSUM") as psS,
            tc.tile_pool(name="psAV", bufs=2, space="PSUM") as psAV,
            tc.tile_pool(name="psP", bufs=2, space="PSUM") as psP,
        ):
            # ---- persistent SBUF ----
            x_sb = persist.tile([128, NEC, SEQ], F16, name="x_sb")
            wq_sb = persist.tile([128, NEC, 128], F16, name="wq_sb")
            wk_sb = persist.tile([128, NEC, 128], F16, name="wk_sb")
            wqk2_sb = persist.tile([128, NEC, 128], F16, name="wqk2_sb")
            wv_sb = persist.tile([128, NEC, DSL], F16, name="wv_sb")
            wp_a = persist.tile([128, EMBED], F16, name="wp_a")
            wp_b = persist.tile([64, EMBED], F16, name="wp_b")
            id_sb = persist.tile([128, 128], F16, name="id_sb")
            # K^T for heads 0,1 (a-scaled), packed at partition halves
            kt01 = persist.tile([128, SEQ], F16, name="kt01")
            # head 2: Q2 at [:,0,:], a*K2 at [:,1,:] -- same partition base
            qk2s = persist.tile([64, 2, SEQ], F16, name="qk2s")
            # V natural layout + ones column: [k-part, kt, head, 65]
            v_sb = persist.tile([128, NKT, NHEAD_CORE, 65], F16, name="v_sb")

            # DMAs ordered by first consumption: K weights + x block 0 first
            nc.sync.dma_start(out=wk_sb[:],
                              in_=wk_d.rearrange("(c p) d -> p c d", p=128))

            def dma_x_chunk(c):
                cc = slice(128 * c, 128 * (c + 1))
                nc.sync.dma_start(
                    out=x_sb[:, :, cc],
                    in_=xT_d[:, cc].rearrange("(c p) s -> p c s", p=128))

            dma_x_chunk(0)
            nc.sync.dma_start(out=wqk2_sb[:],
                              in_=wqk2_d.rearrange("(c p) d -> p c d", p=128))
            dma_x_chunk(1)
            nc.sync.dma_start(out=wv_sb[:],
                              in_=wv_d.rearrange("(c p) d -> p c d", p=128))
            dma_x_chunk(2)
            dma_x_chunk(3)
            for sb in range(1, NQB):
                cols = slice(QB * sb, QB * (sb + 1))
                nc.sync.dma_start(
                    out=x_sb[:, :, cols],
                    in_=xT_d[:, cols].rearrange("(c p) s -> p c s", p=128))
            nc.sync.dma_start(out=wq_sb[:],
                              in_=wq_d.rearrange("(c p) d -> p c d", p=128))
            nc.sync.dma_start(out=wp_a[:], in_=wp_d[0:128, :])
            nc.sync.dma_start(out=wp_b[:], in_=wp_d[128:DSL, :])
            nc.sync.dma_start(out=id_sb[:], in_=id_d[:, :])
            nc.vector.memset(v_sb[:, :, :, 64:65], 1.0)

            # ---- phase 1: K/Q2K2/V projections (psums from psP/psAV) ----
            def phase1_sb(sb, post_qk2=None):
                cols = slice(QB * sb, QB * (sb + 1))
                nsub = 4 if sb == 0 else 1
                kps = psP.tile([128, QB], F32, name="kps", tag="psP")
                qps = psP.tile([128, QB], F32, name="qps", tag="psP")
                for c in range(nsub):
                    sc = slice(QB * sb + 512 // nsub * c,
                               QB * sb + 512 // nsub * (c + 1))
                    oc = slice(512 // nsub * c, 512 // nsub * (c + 1))
                    for e in range(NEC):
                        nc.tensor.matmul(kps[:, oc], wk_sb[:, e, :],
                                         x_sb[:, e, sc],
                                         start=(e == 0), stop=(e == NEC - 1))
                    for e in range(NEC):
                        nc.tensor.matmul(qps[:, oc], wqk2_sb[:, e, :],
                                         x_sb[:, e, sc],
                                         start=(e == 0), stop=(e == NEC - 1))
                    if nsub == 4:
                        kt_abs = 4 * sb + c
                        scs = slice(128 * kt_abs, 128 * (kt_abs + 1))
                        vps = psAV.tile([128, DSL], F32, name="vps",
                                        tag="psAV")
                        for e in range(NEC):
                            nc.tensor.matmul(vps[:], x_sb[:, e, scs],
                                             wv_sb[:, e, :],
                                             start=(e == 0),
                                             stop=(e == NEC - 1))
                        nc.scalar.copy(
                            v_sb[:, kt_abs, :, 0:64],
                            vps[:].rearrange("p (h d) -> p h d",
                                             h=NHEAD_CORE))
                nc.scalar.copy(kt01[:, cols], kps[:])
                nc.scalar.copy(qk2s[:, 0, cols], qps[0:64, :])
                nc.scalar.copy(qk2s[:, 1, cols], qps[64:128, :])
                if post_qk2 is not None:
                    post_qk2(sb)
                if nsub == 1:
                    for c in range(4):  # V s-chunks of 128
                        kt_abs = 4 * sb + c
                        scs = slice(128 * kt_abs, 128 * (kt_abs + 1))
                        vps = psAV.tile([128, DSL], F32, name="vps",
                                        tag="psAV")
                        for e in range(NEC):
                            nc.tensor.matmul(vps[:], x_sb[:, e, scs],
                                             wv_sb[:, e, :],
                                             start=(e == 0),
                                             stop=(e == NEC - 1))
                        nc.scalar.copy(
                            v_sb[:, kt_abs, :, 0:64],
                            vps[:].rearrange("p (h d) -> p h d",
                                             h=NHEAD_CORE))

            # ---- phase 2: attention + projection ----
            # Head-phase pipeline: during head-block p's 32 score+exp steps,
            # head-block p-1's attn@V runs as four per-q-chunk accumulation
            # chains. Each chain's 32 matmuls are emitted contiguously (PSUM
            # accumulation chains within one bank must not interleave with
            # other chains in that bank; cross-bank interleave is fine), 16
            # per step over steps 0..7. exp writes a per-head es buffer
            # [128, 32, 512] (double buffered) so attn@V reads a completed
            # buffer with a full phase of slack.
            HS = [2, 0, 1]
            NPH = NQB * NHEAD_CORE      # 24 head-blocks
            DEFER = 6
            PDEFER = 4

            qt_tiles = {}
            attT_tiles = {}
            _dr = [None]
            av_tiles = {}
            es_bufs = {}
            pend = {}

            def blk(p):
                return p // NHEAD_CORE, HS[p % NHEAD_CORE]

            def emit_qproj(qb):
                qcols = slice(QB * qb, QB * (qb + 1))
                qps = psP.tile([128, QB], F32, name="qps2", tag="psP")
                for e in range(NEC):
                    nc.tensor.matmul(qps[:], wq_sb[:, e, :],
                                     x_sb[:, e, qcols],
                                     start=(e == 0), stop=(e == NEC - 1))
                qt = qtp.tile([128, QB], F16, name="qt", tag="qt")
                nc.scalar.copy(qt[:], qps[:])
                qt_tiles[qb] = qt

            def emit_scores_exp(p, kt):
                qb, h = blk(p)
                qcols = slice(QB * qb, QB * (qb + 1))
                if kt == 0:
                    es_bufs[p] = esp.tile([128, NKT, QB], F16, name="esb",
                                          tag="es")
                sps = psS.tile([128, QB], F32, name="sps", tag="psS")
                kk = slice(128 * kt, 128 * (kt + 1))
                if h < 2:
                    hp = slice(64 * h, 64 * (h + 1))
                    nc.tensor.matmul(sps[:], kt01[hp, kk],
                                     qt_tiles[qb][hp, :],
                                     start=True, stop=True)
                else:
                    nc.tensor.matmul(sps[:], qk2s[:, 1, kk],
                                     qk2s[:, 0, qcols],
                                     start=True, stop=True)
                dst = es_bufs[p][:, kt, :]
                if kt % 2 == 0 and not _ALL_ACT:
                    nc.vector.tensor_scalar(
                        out=dst.bitcast(I16), in0=sps[:],
                        scalar1=SCH_BIAS, scalar2=None, op0=ADD)
                else:
                    nc.scalar.activation(out=dst, in_=sps[:],
                                         func=EXP, scale=ACT_SCALE)

            # AV chain schedules: SCHED[k] = [(qc, kt), ...] per step.
            # Chains stay contiguous per qc; spread over 28 steps so the es
            # buffer frees early, or bunched over 8 steps for the drain phase.
            def _mk_sched(bounds):
                sched = [[] for _ in range(NKT)]
                nsteps = len(bounds) - 1
                for qc in range(4):
                    for j in range(nsteps):
                        for kt in range(bounds[j], bounds[j + 1]):
                            sched[nsteps * qc + j].append((qc, kt))
                return sched

            SCHED_MAIN = _mk_sched([0, 5, 10, 15, 20, 24, 28, 32])
            SCHED_DRAIN = _mk_sched([0, 8, 16, 24, 32])

            def emit_av(p, k, sched):
                if not sched[k]:
                    return
                qb, h = blk(p)
                if k == 0:
                    av_tiles[p] = psAV.tile([128, 4, 65], F32,
                                            name="av", tag="psAV")
                av = av_tiles[p]
                esb = es_bufs[p]
                for qc, kt in sched[k]:
                    nc.tensor.matmul(
                        av[:, qc, :], esb[:, kt, 128 * qc:128 * (qc + 1)],
                        v_sb[:, kt, h, :],
                        start=(kt == 0), stop=(kt == NKT - 1),
                        skip_group_check=True)

            def emit_norm(p):
                qb, h = blk(p)
                av = av_tiles.pop(p)
                del es_bufs[p]
                rec = recp.tile([128, 4], F32, name="rec", tag="rec")
                nc.vector.reciprocal(out=rec[:, :], in_=av[:, :, 64])
                attq = attqp.tile([128, 4, DHEAD], F16, name="attq",
                                  tag="attq")
                for qc in range(4):
                    if qc % 2 == 0:
                        nc.scalar.mul(attq[:, qc, :], av[:, qc, 0:64],
                                      rec[:, qc:qc + 1])
                    else:
                        nc.vector.tensor_scalar(
                            out=attq[:, qc, :], in0=av[:, qc, 0:64],
                            scalar1=rec[:, qc:qc + 1], scalar2=None, op0=MULT)
                return attq

            def emit_transposes(p, attq):
                qb, h = blk(p)
                if h == HS[0]:
                    attT_tiles[qb] = (
                        attTp.tile([128, QB], F16, name="attT01", tag="a01"),
                        attTp.tile([64, QB], F16, name="attT2", tag="a2"))
                attT01, attT2 = attT_tiles[qb]
                tp = psP.tile([64, 4, 128], F16, name="tp", tag="psP")
                for qc in range(4):
                    nc.tensor.transpose(tp[:, qc, :], attq[:, qc, :],
                                        id_sb[:])
                if h == 0:
                    dst = attT01[0:64, :]
                elif h == 1:
                    dst = attT01[64:128, :]
                else:
                    dst = attT2[:, :]
                nc.scalar.copy(dst.rearrange("p (c q) -> p c q", c=4), tp[:])

            def emit_proj_f(qb, f, drain=False):
                qcols = slice(QB * qb, QB * (qb + 1))
                attT01, attT2 = attT_tiles[qb]
                fc = slice(128 * f, 128 * (f + 1))
                if drain:
                    yps = psS.tile([128, QB], F32, name="yps", tag="psS")
                else:
                    yps = psP.tile([128, QB], F32, name="yps", tag="psP")
                nc.tensor.matmul(yps[:], wp_a[:, fc], attT01[:],
                                 start=True, stop=False)
                nc.tensor.matmul(yps[:], wp_b[:, fc], attT2[:],
                                 start=False, stop=True)
                if drain:
                    if f % 2 == 0:
                        _dr[0] = ysbp.tile([128, 2, QB], F16, name="ysb2",
                                           tag="ysb2")
                    ysb2 = _dr[0]
                    if f % 2 == 0:
                        nc.scalar.copy(ysb2[:, 0, :], yps[:])
                    else:
                        nc.vector.tensor_copy(ysb2[:, 1, :], yps[:])
                        fc2 = slice(128 * (f - 1), 128 * (f + 1))
                        nc.sync.dma_start(
                            out=yT_d[fc2, qcols].rearrange(
                                "(c p) q -> p c q", p=128),
                            in_=ysb2[:])
                else:
                    ysb = ysbp.tile([128, QB], F16, name="ysb", tag="ysb")
                    if f % 2 == 0:
                        nc.scalar.copy(ysb[:], yps[:])
                    else:
                        nc.vector.tensor_copy(ysb[:], yps[:])
                    nc.sync.dma_start(out=yT_d[fc, qcols], in_=ysb[:])
                if f == NEC - 1:
                    attT_tiles.pop(qb)

            def p0_scores(sb):
                for kt in range(4 * sb, 4 * sb + 4):
                    emit_scores_exp(0, kt)

            for sb in range(NQB):
                phase1_sb(sb, post_qk2=p0_scores)
            emit_qproj(0)
            for g in range(NKT, (NPH + 1) * NKT):
                p, k = divmod(g, NKT)
                if p < NPH:
                    emit_scores_exp(p, k)
                    if False:
                        pass  # qproj for next qb emitted below at own time
                    if p % NHEAD_CORE == 0 and k == 18:
                        nqb = p // NHEAD_CORE
                        if nqb not in qt_tiles:
                            emit_qproj(nqb)
                if p >= 1:
                    if p == NPH:
                        if k < 16:
                            emit_av(p - 1, k, SCHED_DRAIN)
                        if k == 16:
                            pend[g + 1] = ('tp', p - 1, emit_norm(p - 1))
                    else:
                        emit_av(p - 1, k, SCHED_MAIN)
                        if k == 28:
                            pend[g + DEFER] = ('tp', p - 1,
                                               emit_norm(p - 1))
                if g in pend:
                    item = pend.pop(g)
                    if item[0] == 'tp':
                        _, pp, attq = item
                        emit_transposes(pp, attq)
                        qb, h = blk(pp)
                        if h == HS[-1]:
                            for f in range(NEC):
                                pend[g + PDEFER + 2 * f] = ('proj', qb, f)
                    else:
                        emit_proj_f(item[1], item[2], drain=(item[1] == NQB - 1))
            for gg in sorted(pend):
                item = pend[gg]
                if item[0] == 'tp':
                    _, pp, attq = item
                    emit_transposes(pp, attq)
                    qb, h = blk(pp)
                    if h == HS[-1]:
                        for f in range(NEC):
                            emit_proj_f(qb, f)
                else:
                    emit_proj_f(item[1], item[2], drain=True)

    nc.compile()
    return nc


def _get_nc():
    if "nc" not in _CACHED:
        _CACHED["nc"] = _build()
    return _CACHED["nc"]


def _make_in_maps(x, W_qkv, W_proj):
    f16 = np.float16
    ident = np.eye(128, dtype=f16)
    in_maps = []
    for c in range(8):
        b = c // 4
        g = c % 4
        sl = slice(DSL * g, DSL * (g + 1))
        xT = np.ascontiguousarray(x[b].T).astype(f16)
        wqT = np.ascontiguousarray(W_qkv[0:EMBED][sl, :].T)          # [768,192]
        wkT = np.ascontiguousarray(W_qkv[EMBED:2 * EMBED][sl, :].T) * A_FOLD
        wvT = np.ascontiguousarray(W_qkv[2 * EMBED:3 * EMBED][sl, :].T)
        wp = np.ascontiguousarray(W_proj[:, sl].T)                   # [192,768]
        wqk2 = np.concatenate([wqT[:, 128:192], wkT[:, 128:192]], axis=1)
        in_maps.append({
            "xT": xT,
            "wq": wqT[:, 0:128].astype(f16),
            "wk": wkT[:, 0:128].astype(f16),
            "wqk2": np.ascontiguousarray(wqk2).astype(f16),
            "wv": wvT.astype(f16),
            "wp": wp.astype(f16),
            "ident": ident,
        })
    return in_maps


def kernel(x, W_qkv, W_proj, b_proj):
    from concourse.bass_utils import run_bass_kernel_spmd

    x = np.asarray(x, dtype=np.float32)
    W_qkv = np.asarray(W_qkv, dtype=np.float32)
    W_proj = np.asarray(W_proj, dtype=np.float32)
    b_proj = np.asarray(b_proj, dtype=np.float32)

    nc = _get_nc()
    in_maps = _make_in_maps(x, W_qkv, W_proj)
    res = run_bass_kernel_spmd(nc, in_maps, core_ids=list(range(8)))

    y = np.zeros((2, SEQ, EMBED), dtype=np.float32)
    for c in range(8):
        y[c // 4] += res.results[c]["yT"].astype(np.float32).T
    y += b_proj
    return y
